# revision 24
# baseline (speedup 1.0000x reference)
"""Fused QK-linear attention kernel for 8 TRN2 NeuronCores (Bass/Tile).

Computes, per batch b (one batch per core):
    q = x @ Wq^T ; k = x @ Wk^T
    sim  = (q @ k^T) / sqrt(d)
    attn = softmax(sim, axis=-1)
    out  = attn @ x

Math used on device: sim = x @ P @ x^T with P = (Wq^T @ Wk) / sqrt(d)
(host-precomputed; every folded scale is a power of two, exact). Softmax
is computed without per-row max subtraction, but with a constant shift
folded into the exp bias (softmax is exactly shift-invariant) so exp()
stays inside fp8 range.

All big matmuls run as fp8e4m3 DoubleRow (K=256 per instruction, 0.5
cyc/row: 4x the fp32r MAC rate). fp8's ~2.2% quantization error is
suppressed below the accuracy gate by hi+lo fp8 splitting of every
matmul operand except exp(sim) itself:
    x  -> xh + xl        (host split, both fp8)
    P' = 128*P -> Ph + Pl  (host split)
    w' = x @ P'          (device, 3 DoubleRow chains/psum: xh*Ph + xh*Pl
                          + xl*Ph, exact fp32 PSUM) -> wh + wl split on
                          DVE (ec0) and Pool (ec1)
    simT' = x @ w'^T     (3 chains: xh*wh + xl*wh + xh*wl) = 128*sim[j,i]
    ET  = fp8(exp(simT' * 2^-7 - 8*ln2))   (ScalarE, PSUM->SBUF)
    num = ET^T @ [xh|1|0] + ET^T @ [xl|0|0]  (col 256 = rowsum)
    out = num[:, :256] / num[:, 256]
Measured end-to-end rel err (L2) vs the fp32 reference: ~1.6e-2.

Scheduling: phase-1 psum groups are interleaved into phase 2's stream so
the PE never stalls on the DVE/Pool w-split; out-chunks of block ib-1
interleave into block ib (baseline-style software pipeline); input DMAs
are packed per-tensor (hi+lo together) and split across the HWDGE (sync)
and SWDGE (pool) paths, as are the output stores.
"""

import numpy as np

_B, _N, _D = 8, 2048, 256
_P = 128
_NJC = _N // _P        # 16 chunks of 128 along sequence (j)
_DCH = _D // _P        # 2 chunks of 128 along feature dim
_IBLK = 512            # i-block (matmul moving free dim)
_XW = _D + 2           # x_aug width: ones col at _D, zero pad at _D+1
_NIB = _N // _IBLK     # 4
_ICH = _IBLK // _P     # 4 i-chunks of 128 per i-block
_EXPB = float(-8.0 * np.log(2.0))
_SIMSCALE = 2.0 ** -7

_nc_cache = {}


def _build_program(loop_iters: int = 1):
    from contextlib import ExitStack, nullcontext
    from concourse import bacc, tile, mybir

    f32 = mybir.dt.float32
    fp8 = mybir.dt.float8e4
    DR = mybir.MatmulPerfMode.DoubleRow
    act_exp = mybir.ActivationFunctionType.Exp

    nc = bacc.Bacc("TRN2", debug=False, enable_asserts=True, num_devices=_B)
    # inputs pre-arranged on host to SBUF layouts (partition-major, hi/lo
    # packed into one tensor each so one DMA covers both)
    P_d = nc.dram_tensor("P", [_P, 2, _DCH, _D], fp8, kind="ExternalInput").ap()
    xT_d = nc.dram_tensor("xT", [_P, 2, _DCH, _N], fp8, kind="ExternalInput").ap()
    xa_d = nc.dram_tensor("xa", [_P, _NJC, 2, _XW], fp8, kind="ExternalInput").ap()
    out_d = nc.dram_tensor("out", [_N, _D], f32, kind="ExternalOutput").ap()

    with ExitStack() as ctx:
        tc = ctx.enter_context(tile.TileContext(nc))
        consts = ctx.enter_context(tc.tile_pool(name="consts", bufs=1))
        etp = ctx.enter_context(tc.tile_pool(name="et", bufs=2))
        outp = ctx.enter_context(tc.tile_pool(name="outsb", bufs=4))
        smallp = ctx.enter_context(tc.tile_pool(name="small", bufs=4))
        # pss: 2-bank psums for phase-2 jc-pairs; pso: 1-bank psums for
        # warmup + phase-1 groups + phase-3 out chunks
        pss = ctx.enter_context(tc.tile_pool(name="pss", bufs=2, space="PSUM"))
        pso = ctx.enter_context(tc.tile_pool(name="pso", bufs=4, space="PSUM"))

        P_sb = consts.tile([_P, 2, _DCH, _D], fp8)
        xT_sb = consts.tile([_P, 2, _DCH, _N], fp8)
        xa_sb = consts.tile([_P, _NJC, 2, _XW], fp8)
        wh_sb = consts.tile([_P, _DCH, _N], fp8)
        wl_sb = consts.tile([_P, _DCH, _N], fp8)
        bias_sb = consts.tile([_P, 1], f32)
        nc.vector.memset(bias_sb, _EXPB)

        # PE warmup while the input DMAs are in flight: keeps the PE busy from
        # t=0 and gets the HAM clock gate to 2.4 GHz before real work starts.
        warm = consts.tile([_P, 2 * _P], mybir.dt.bfloat16)
        nc.gpsimd.memset(warm, 0.0)

        # DMA_ENGINES is effectively serial across transfers: order inputs by
        # first use (P, xT blocks, then the big xa which isn't needed until
        # the first out-chunk at ~11us).
        nc.sync.dma_start(out=P_sb, in_=P_d)
        for nb in range(_NIB):
            sl = slice(nb * _IBLK, (nb + 1) * _IBLK)
            for hl in range(2):
                nc.sync.dma_start(out=xT_sb[:, hl, :, sl],
                                  in_=xT_d[:, hl, :, sl])
            if nb == 2:
                # xa is needed by the first out-chunk (~10us); xT block 3
                # not until phase 1's last group (~12us)
                nc.sync.dma_start(out=xa_sb, in_=xa_d)

        # A couple of tiny matmuls anchor the PE clock-ramp timer early; the
        # ramp completes on wall-time, so no long busy-warmup is needed.
        for _ in range(3):
            ps = pso.tile([_P, _IBLK], f32, tag="po")
            nc.tensor.matmul(out=ps[:, 0:2 * _P], lhsT=warm[:, 0:_P], rhs=warm,
                             start=True, stop=True)

        loop_cm = tc.For_i(0, loop_iters, 1) if loop_iters > 1 else nullcontext()
        ctx.enter_context(loop_cm)

        def p1_group(nb, use_pss=False):
            """w'[e, i-block nb]: one 1-bank psum per ec chunk; wh+wl split
            on DVE (copies first so phase 2's wh-chains unblock early).
            use_pss places both ec halves in one 2-bank pss tile (for the
            late group, once the pso pool is serving out-chunks)."""
            isl = slice(nb * _IBLK, (nb + 1) * _IBLK)
            pshalf = []
            big = None
            if use_pss:
                big = pss.tile([_P, 2 * _IBLK], f32, tag="sim", name="p1big")
            for ec in range(_DCH):
                esl = slice(ec * _P, (ec + 1) * _P)
                if use_pss:
                    ps = big[:, ec * _IBLK:(ec + 1) * _IBLK]
                else:
                    ps = pso.tile([_P, _IBLK], f32, tag="po")
                pshalf.append(ps)
                nc.tensor.matmul(out=ps, lhsT=P_sb[:, 0, :, esl],
                                 rhs=xT_sb[:, 0, :, isl], start=True,
                                 stop=False, perf_mode=DR)
                nc.tensor.matmul(out=ps, lhsT=P_sb[:, 1, :, esl],
                                 rhs=xT_sb[:, 0, :, isl], start=False,
                                 stop=False, perf_mode=DR)
                nc.tensor.matmul(out=ps, lhsT=P_sb[:, 0, :, esl],
                                 rhs=xT_sb[:, 1, :, isl], start=False,
                                 stop=True, perf_mode=DR)
                nc.vector.tensor_copy(out=wh_sb[:, ec, isl], in_=ps)
            for ec in range(_DCH):
                nc.vector.tensor_sub(out=wl_sb[:, ec, isl], in0=pshalf[ec],
                                     in1=wh_sb[:, ec, isl])

        out_r = out_d.rearrange("(g p) d -> p g d", p=_P)

        def p2_pair(ib, jp, et):
            """simT' + exp for j-chunks (2jp, 2jp+1) of i-block ib."""
            isl = slice(ib * _IBLK, (ib + 1) * _IBLK)
            ps = pss.tile([_P, 2 * _IBLK], f32, tag="sim")
            for h in range(2):
                jsl = slice((2 * jp + h) * _P, (2 * jp + h + 1) * _P)
                half = ps[:, h * _IBLK:(h + 1) * _IBLK]
                nc.tensor.matmul(out=half, lhsT=xT_sb[:, 0, :, jsl],
                                 rhs=wh_sb[:, :, isl], start=True,
                                 stop=False, perf_mode=DR)
                nc.tensor.matmul(out=half, lhsT=xT_sb[:, 1, :, jsl],
                                 rhs=wh_sb[:, :, isl], start=False,
                                 stop=False, perf_mode=DR)
                nc.tensor.matmul(out=half, lhsT=xT_sb[:, 0, :, jsl],
                                 rhs=wl_sb[:, :, isl], start=False,
                                 stop=True, perf_mode=DR)
            nc.scalar.activation(out=et[:, 2 * jp:2 * jp + 2, :], in_=ps,
                                 func=act_exp, scale=_SIMSCALE, bias=bias_sb)

        def chunk_dr_pair(et, po, t, u):
            """the j-pair-u contribution (hi + lo chains) to chunk t's psum."""
            poa = po[:, 0:_XW]
            tsl = slice(t * _P, (t + 1) * _P)
            nc.tensor.matmul(out=poa, lhsT=et[:, 2 * u:2 * u + 2, tsl],
                             rhs=xa_sb[:, 2 * u:2 * u + 2, 0, :],
                             start=(u == 0), stop=False, perf_mode=DR)
            nc.tensor.matmul(out=poa, lhsT=et[:, 2 * u:2 * u + 2, tsl],
                             rhs=xa_sb[:, 2 * u:2 * u + 2, 1, :],
                             start=False, stop=(u == _NJC // 2 - 1),
                             perf_mode=DR)

        act_copy = mybir.ActivationFunctionType.Copy

        def finish_chunk(ib, po, t, scalar_mul=False):
            """normalize chunk t of block ib and store it. scalar_mul moves
            the multiply to ScalarE (idle at the tail) so the last chunks'
            normalizations pipeline instead of serializing on DVE."""
            poa = po[:, 0:_XW]
            recip = smallp.tile([_P, 1], f32)
            nc.vector.reciprocal(out=recip, in_=poa[:, _D:_D + 1])
            o_t = outp.tile([_P, _D], f32)
            if scalar_mul:
                nc.scalar.activation(out=o_t, in_=poa[:, 0:_D], func=act_copy,
                                     scale=recip)
            else:
                nc.vector.tensor_scalar_mul(out=o_t, in0=poa[:, 0:_D],
                                            scalar1=recip)
            chunk = ib * _ICH + t
            eng = nc.sync if chunk % 2 == 0 else nc.gpsimd
            eng.dma_start(out=out_r[:, chunk, :], in_=o_t)

        def out_chunk(ib, et, t):
            """one i-chunk of the numerator/rowsum + normalize + store."""
            po = pso.tile([_P, _IBLK], f32, tag="po")
            for u in range(_NJC // 2):
                chunk_dr_pair(et, po, t, u)
            finish_chunk(ib, po, t)

        # Software pipeline: p1(nb0/nb1) prologue, p1(nb2) early in block 0,
        # p1(nb3) after block 1 (so its DVE ops queue AFTER block-0 norms);
        # out-chunks of block ib-1 interleave into block ib; tail runs block
        # 3's out-chunks.
        p1_group(0)
        p1_group(1)
        prev = None
        for ib in range(_NIB - 1):
            et = etp.tile([_P, _NJC, _IBLK], fp8, tag="et")
            for jp in range(_NJC // 2):
                p2_pair(ib, jp, et)
                if ib == 0:
                    if jp == 1:
                        p1_group(2)
                elif 3 <= jp <= 6:
                    pib, pet = prev
                    out_chunk(pib, pet, jp - 3)
            if ib == 1:
                p1_group(3)
            prev = (ib, et)

        # Last block: block 2's chunks run in the first half. Block 3's chunk
        # t0 accumulates incrementally through the block's second half (so its
        # store launches right after the last activation); t1-t3 run as full
        # trailing chains whose PE time hides t0-t2's normalize+DMA latency.
        pib, pet = prev
        et = etp.tile([_P, _NJC, _IBLK], fp8, tag="et")
        po0 = None
        for jp in range(_NJC // 2):
            p2_pair(_NIB - 1, jp, et)
            if jp <= 3:
                out_chunk(pib, pet, jp)
            elif jp == 4:
                po0 = pso.tile([_P, _IBLK], f32, tag="po")
                for u in range(5):
                    chunk_dr_pair(et, po0, 0, u)
            else:
                chunk_dr_pair(et, po0, 0, jp)
        finish_chunk(_NIB - 1, po0, 0, scalar_mul=True)
        for t in range(1, _ICH):
            out_chunk(_NIB - 1, et, t)

    nc.compile()
    return nc


def _get_nc(mm_dtype=None):
    key = "fp8dr"
    if key not in _nc_cache:
        _nc_cache[key] = _build_program()
    return _nc_cache[key]


def _prep_inputs(x, Wq, Wk):
    import ml_dtypes

    E4 = ml_dtypes.float8_e4m3
    x = np.asarray(x, dtype=np.float32)
    Wq = np.asarray(Wq, dtype=np.float32)
    Wk = np.asarray(Wk, dtype=np.float32)
    # P' = 128 * Wq^T @ Wk / sqrt(d); hi+lo fp8 split (host)
    P = ((Wq.astype(np.float64).T @ Wk.astype(np.float64)) * (2.0 ** -4) * 128.0
         ).astype(np.float32)
    Ph8 = P.astype(E4)
    Pl8 = (P - Ph8.astype(np.float32)).astype(E4)
    # [p, hl, dc, e]: P'_hl[128*dc + p, e]
    Pp = np.stack([Ph8.reshape(_DCH, _P, _D).transpose(1, 0, 2),
                   Pl8.reshape(_DCH, _P, _D).transpose(1, 0, 2)], axis=1)
    Pp = np.ascontiguousarray(Pp)

    xh8 = x.astype(E4)                                   # [b, n, d]
    xl8 = (x - xh8.astype(np.float32)).astype(E4)
    # xT layout [b, p, hl, dc, n]: x_hl[n, 128*dc + p]
    xTh = xh8.transpose(0, 2, 1).reshape(_B, _DCH, _P, _N)
    xTl = xl8.transpose(0, 2, 1).reshape(_B, _DCH, _P, _N)
    xT = np.empty((_B, _P, 2, _DCH, _N), E4)
    xT[:, :, 0, :, :] = xTh.transpose(0, 2, 1, 3)
    xT[:, :, 1, :, :] = xTl.transpose(0, 2, 1, 3)
    # x_aug layout [b, p, t, hl, e]: x_hl[128*t + p, e], col _D = ones (hi)
    xa = np.zeros((_B, _P, _NJC, 2, _XW), E4)
    xa[:, :, :, 0, 0:_D] = xh8.reshape(_B, _NJC, _P, _D).transpose(0, 2, 1, 3)
    xa[:, :, :, 1, 0:_D] = xl8.reshape(_B, _NJC, _P, _D).transpose(0, 2, 1, 3)
    xa[:, :, :, 0, _D] = np.float32(1.0)

    in_maps = [
        {"xT": np.ascontiguousarray(xT[b]),
         "xa": np.ascontiguousarray(xa[b]),
         "P": Pp}
        for b in range(_B)
    ]
    return in_maps


def _run_on_hw(nc, in_maps, trace=False):
    from concourse import bass_utils
    from concourse.bass_interp import get_hw_module

    old_m = nc.m
    nc.m = get_hw_module(nc.m)
    try:
        res = bass_utils.run_bass_kernel_spmd(
            nc, in_maps, core_ids=list(range(len(in_maps))), trace=trace
        )
    finally:
        nc.m = old_m
    return res


def kernel(x, Wq, Wk):
    in_maps = _prep_inputs(x, Wq, Wk)
    nc = _get_nc()
    res = _run_on_hw(nc, in_maps)
    out = np.stack([res.results[b]["out"] for b in range(_B)], axis=0)
    return np.ascontiguousarray(out.astype(np.float32))


# revision 29
# speedup vs baseline: 1.0114x; 1.0114x over previous
"""Fused QK-linear attention kernel for 8 TRN2 NeuronCores (Bass/Tile).

Computes, per batch b (one batch per core):
    q = x @ Wq^T ; k = x @ Wk^T
    sim  = (q @ k^T) / sqrt(d)
    attn = softmax(sim, axis=-1)
    out  = attn @ x

Math used on device: sim = x @ P @ x^T with P = (Wq^T @ Wk) / sqrt(d)
(host-precomputed; every folded scale is a power of two, exact). Softmax
is computed without per-row max subtraction, but with a constant shift
folded into the exp bias (softmax is exactly shift-invariant) so exp()
stays inside fp8 range.

All big matmuls run as fp8e4m3 DoubleRow (K=256 per instruction, 0.5
cyc/row: 4x the fp32r MAC rate). fp8's ~2.2% quantization error is
suppressed below the accuracy gate by hi+lo fp8 splitting of every
matmul operand except exp(sim) itself:
    x  -> xh + xl        (host split, both fp8)
    P' = 128*P -> Ph + Pl  (host split)
    w' = x @ P'          (device, 3 DoubleRow chains/psum: xh*Ph + xh*Pl
                          + xl*Ph, exact fp32 PSUM) -> wh + wl split on DVE
    simT' = x @ w'^T     (3 chains: xh*wh + xl*wh + xh*wl) = 128*sim[j,i]
    ET  = fp8(exp(simT' * 2^-7 - 8*ln2))   (ScalarE, PSUM->SBUF)
    num = ET^T @ [xh|1|0] + ET^T @ [xl|0|0]  (col 256 = rowsum)
    out = num[:, :256] / num[:, 256]
Measured end-to-end rel err (L2) vs the fp32 reference: ~1.6e-2.

Scheduling: phase-1 psum groups are interleaved into phase 2's stream at
the DVE w-split drain pace; out-chunks of block ib-1 interleave into
block ib (software pipeline); block 3's chunk t0 accumulates
incrementally so its store launches right after the last activation,
while chunks t1-t3 trail and hide the earlier stores' DMA latency;
input DMAs are ordered by first use (DMA transfers are serial), and
output stores alternate between the HWDGE (sync) and SWDGE (pool) DMA
paths.
"""

import numpy as np

_B, _N, _D = 8, 2048, 256
_P = 128
_NJC = _N // _P        # 16 chunks of 128 along sequence (j)
_DCH = _D // _P        # 2 chunks of 128 along feature dim
_IBLK = 512            # i-block (matmul moving free dim)
_XW = _D + 2           # x_aug width: ones col at _D, zero pad at _D+1
_NIB = _N // _IBLK     # 4
_ICH = _IBLK // _P     # 4 i-chunks of 128 per i-block
_EXPB = float(-8.0 * np.log(2.0))
_SIMSCALE = 2.0 ** -7

_nc_cache = {}


def _build_program(loop_iters: int = 1):
    from contextlib import ExitStack, nullcontext
    from concourse import bacc, tile, mybir

    f32 = mybir.dt.float32
    fp8 = mybir.dt.float8e4
    DR = mybir.MatmulPerfMode.DoubleRow
    act_exp = mybir.ActivationFunctionType.Exp

    nc = bacc.Bacc("TRN2", debug=False, enable_asserts=True, num_devices=_B)
    # inputs pre-arranged on host to SBUF layouts (partition-major, hi/lo
    # packed into one tensor each so one DMA covers both)
    P_d = nc.dram_tensor("P", [_P, 2, _DCH, _D], fp8, kind="ExternalInput").ap()
    xT_d = nc.dram_tensor("xT", [_P, 2, _DCH, _N], fp8, kind="ExternalInput").ap()
    xa_d = nc.dram_tensor("xa", [_P, _NJC, 2, _XW], fp8, kind="ExternalInput").ap()
    out_d = nc.dram_tensor("out", [_N, _D], f32, kind="ExternalOutput").ap()

    with ExitStack() as ctx:
        tc = ctx.enter_context(tile.TileContext(nc))
        consts = ctx.enter_context(tc.tile_pool(name="consts", bufs=1))
        etp = ctx.enter_context(tc.tile_pool(name="et", bufs=2))
        outp = ctx.enter_context(tc.tile_pool(name="outsb", bufs=4))
        smallp = ctx.enter_context(tc.tile_pool(name="small", bufs=4))
        # pss: 2-bank psums for phase-2 jc-pairs; pso: 1-bank psums for
        # warmup + phase-1 groups + phase-3 out chunks
        pss = ctx.enter_context(tc.tile_pool(name="pss", bufs=2, space="PSUM"))
        pso = ctx.enter_context(tc.tile_pool(name="pso", bufs=4, space="PSUM"))

        P_sb = consts.tile([_P, 2, _DCH, _D], fp8)
        xT_sb = consts.tile([_P, 2, _DCH, _N], fp8)
        xa_sb = consts.tile([_P, _NJC, 2, _XW], fp8)
        wh_sb = consts.tile([_P, _DCH, _N], fp8)
        wl_sb = consts.tile([_P, _DCH, _N], fp8)
        bias_sb = consts.tile([_P, 1], f32)
        nc.vector.memset(bias_sb, _EXPB)

        # PE warmup while the input DMAs are in flight: keeps the PE busy from
        # t=0 and gets the HAM clock gate to 2.4 GHz before real work starts.
        warm = consts.tile([_P, 2 * _P], mybir.dt.bfloat16)
        nc.gpsimd.memset(warm, 0.0)

        # DMA_ENGINES is effectively serial across transfers: order inputs by
        # first use (P, xT blocks, then the big xa which isn't needed until
        # the first out-chunk at ~11us).
        nc.sync.dma_start(out=P_sb, in_=P_d)
        for nb in range(_NIB):
            sl = slice(nb * _IBLK, (nb + 1) * _IBLK)
            for hl in range(2):
                nc.sync.dma_start(out=xT_sb[:, hl, :, sl],
                                  in_=xT_d[:, hl, :, sl])
            if nb == 2:
                # xa is needed by the first out-chunk (~10us); xT block 3
                # not until phase 1's last group (~12us)
                nc.sync.dma_start(out=xa_sb, in_=xa_d)

        # A couple of tiny matmuls anchor the PE clock-ramp timer early; the
        # ramp completes on wall-time, so no long busy-warmup is needed.
        for _ in range(3):
            ps = pso.tile([_P, _IBLK], f32, tag="po")
            nc.tensor.matmul(out=ps[:, 0:2 * _P], lhsT=warm[:, 0:_P], rhs=warm,
                             start=True, stop=True)

        loop_cm = tc.For_i(0, loop_iters, 1) if loop_iters > 1 else nullcontext()
        ctx.enter_context(loop_cm)

        def p1_group(nb, use_pss=False):
            """w'[e, i-block nb]: one 1-bank psum per ec chunk; wh+wl split
            on DVE (copies first so phase 2's wh-chains unblock early).
            use_pss places both ec halves in one 2-bank pss tile (for the
            late group, once the pso pool is serving out-chunks)."""
            isl = slice(nb * _IBLK, (nb + 1) * _IBLK)
            pshalf = []
            big = None
            if use_pss:
                big = pss.tile([_P, 2 * _IBLK], f32, tag="sim", name="p1big")
            for ec in range(_DCH):
                esl = slice(ec * _P, (ec + 1) * _P)
                if use_pss:
                    ps = big[:, ec * _IBLK:(ec + 1) * _IBLK]
                else:
                    ps = pso.tile([_P, _IBLK], f32, tag="po")
                pshalf.append(ps)
                nc.tensor.matmul(out=ps, lhsT=P_sb[:, 0, :, esl],
                                 rhs=xT_sb[:, 0, :, isl], start=True,
                                 stop=False, perf_mode=DR)
                nc.tensor.matmul(out=ps, lhsT=P_sb[:, 1, :, esl],
                                 rhs=xT_sb[:, 0, :, isl], start=False,
                                 stop=False, perf_mode=DR)
                nc.tensor.matmul(out=ps, lhsT=P_sb[:, 0, :, esl],
                                 rhs=xT_sb[:, 1, :, isl], start=False,
                                 stop=True, perf_mode=DR)
                nc.vector.tensor_copy(out=wh_sb[:, ec, isl], in_=ps)
            for ec in range(_DCH):
                nc.vector.tensor_sub(out=wl_sb[:, ec, isl], in0=pshalf[ec],
                                     in1=wh_sb[:, ec, isl])

        out_r = out_d.rearrange("(g p) d -> p g d", p=_P)

        def p2_pair(ib, jp, et):
            """simT' + exp for j-chunks (2jp, 2jp+1) of i-block ib."""
            isl = slice(ib * _IBLK, (ib + 1) * _IBLK)
            ps = pss.tile([_P, 2 * _IBLK], f32, tag="sim")
            for h in range(2):
                jsl = slice((2 * jp + h) * _P, (2 * jp + h + 1) * _P)
                half = ps[:, h * _IBLK:(h + 1) * _IBLK]
                nc.tensor.matmul(out=half, lhsT=xT_sb[:, 0, :, jsl],
                                 rhs=wh_sb[:, :, isl], start=True,
                                 stop=False, perf_mode=DR)
                nc.tensor.matmul(out=half, lhsT=xT_sb[:, 1, :, jsl],
                                 rhs=wh_sb[:, :, isl], start=False,
                                 stop=False, perf_mode=DR)
                nc.tensor.matmul(out=half, lhsT=xT_sb[:, 0, :, jsl],
                                 rhs=wl_sb[:, :, isl], start=False,
                                 stop=True, perf_mode=DR)
            nc.scalar.activation(out=et[:, 2 * jp:2 * jp + 2, :], in_=ps,
                                 func=act_exp, scale=_SIMSCALE, bias=bias_sb)

        def chunk_dr_pair(et, po, t, u):
            """the j-pair-u contribution (hi + lo chains) to chunk t's psum."""
            poa = po[:, 0:_XW]
            tsl = slice(t * _P, (t + 1) * _P)
            nc.tensor.matmul(out=poa, lhsT=et[:, 2 * u:2 * u + 2, tsl],
                             rhs=xa_sb[:, 2 * u:2 * u + 2, 0, :],
                             start=(u == 0), stop=False, perf_mode=DR)
            nc.tensor.matmul(out=poa, lhsT=et[:, 2 * u:2 * u + 2, tsl],
                             rhs=xa_sb[:, 2 * u:2 * u + 2, 1, :],
                             start=False, stop=(u == _NJC // 2 - 1),
                             perf_mode=DR)

        act_copy = mybir.ActivationFunctionType.Copy

        def finish_chunk(ib, po, t, scalar_mul=False):
            """normalize chunk t of block ib and store it. scalar_mul moves
            the multiply to ScalarE (idle at the tail) so the last chunks'
            normalizations pipeline instead of serializing on DVE."""
            poa = po[:, 0:_XW]
            recip = smallp.tile([_P, 1], f32)
            nc.vector.reciprocal(out=recip, in_=poa[:, _D:_D + 1])
            o_t = outp.tile([_P, _D], f32)
            if scalar_mul:
                nc.scalar.activation(out=o_t, in_=poa[:, 0:_D], func=act_copy,
                                     scale=recip)
            else:
                nc.vector.tensor_scalar_mul(out=o_t, in0=poa[:, 0:_D],
                                            scalar1=recip)
            chunk = ib * _ICH + t
            eng = nc.sync if chunk % 2 == 0 else nc.gpsimd
            eng.dma_start(out=out_r[:, chunk, :], in_=o_t)

        def out_chunk(ib, et, t):
            """one i-chunk of the numerator/rowsum + normalize + store."""
            po = pso.tile([_P, _IBLK], f32, tag="po")
            for u in range(_NJC // 2):
                chunk_dr_pair(et, po, t, u)
            finish_chunk(ib, po, t)

        # Software pipeline: p1(nb0/nb1) prologue, p1(nb2) early in block 0,
        # p1(nb3) after block 1 (so its DVE ops queue AFTER block-0 norms);
        # out-chunks of block ib-1 interleave into block ib; tail runs block
        # 3's out-chunks.
        p1_group(0)
        p1_group(1)
        prev = None
        for ib in range(_NIB - 1):
            et = etp.tile([_P, _NJC, _IBLK], fp8, tag="et")
            for jp in range(_NJC // 2):
                p2_pair(ib, jp, et)
                if ib == 0:
                    if jp == 1:
                        p1_group(2)
                elif 4 <= jp <= 7:
                    pib, pet = prev
                    out_chunk(pib, pet, jp - 4)
            if ib == 1:
                p1_group(3)
            prev = (ib, et)

        # Last block: block 2's chunks run in the first half. Block 3's chunk
        # t0 accumulates incrementally through the block's second half (so its
        # store launches right after the last activation); t1-t3 run as full
        # trailing chains whose PE time hides t0-t2's normalize+DMA latency.
        pib, pet = prev
        et = etp.tile([_P, _NJC, _IBLK], fp8, tag="et")
        po0 = None
        for jp in range(_NJC // 2):
            p2_pair(_NIB - 1, jp, et)
            if jp <= 3:
                out_chunk(pib, pet, jp)
            elif jp == 4:
                po0 = pso.tile([_P, _IBLK], f32, tag="po")
                for u in range(5):
                    chunk_dr_pair(et, po0, 0, u)
            else:
                chunk_dr_pair(et, po0, 0, jp)
        finish_chunk(_NIB - 1, po0, 0, scalar_mul=True)
        for t in range(1, _ICH):
            out_chunk(_NIB - 1, et, t)

    nc.compile()
    return nc


def _get_nc(mm_dtype=None):
    key = "fp8dr"
    if key not in _nc_cache:
        _nc_cache[key] = _build_program()
    return _nc_cache[key]


def _prep_inputs(x, Wq, Wk):
    import ml_dtypes

    E4 = ml_dtypes.float8_e4m3
    x = np.asarray(x, dtype=np.float32)
    Wq = np.asarray(Wq, dtype=np.float32)
    Wk = np.asarray(Wk, dtype=np.float32)
    # P' = 128 * Wq^T @ Wk / sqrt(d); hi+lo fp8 split (host)
    P = ((Wq.astype(np.float64).T @ Wk.astype(np.float64)) * (2.0 ** -4) * 128.0
         ).astype(np.float32)
    Ph8 = P.astype(E4)
    Pl8 = (P - Ph8.astype(np.float32)).astype(E4)
    # [p, hl, dc, e]: P'_hl[128*dc + p, e]
    Pp = np.stack([Ph8.reshape(_DCH, _P, _D).transpose(1, 0, 2),
                   Pl8.reshape(_DCH, _P, _D).transpose(1, 0, 2)], axis=1)
    Pp = np.ascontiguousarray(Pp)

    xh8 = x.astype(E4)                                   # [b, n, d]
    xl8 = (x - xh8.astype(np.float32)).astype(E4)
    # xT layout [b, p, hl, dc, n]: x_hl[n, 128*dc + p]
    xTh = xh8.transpose(0, 2, 1).reshape(_B, _DCH, _P, _N)
    xTl = xl8.transpose(0, 2, 1).reshape(_B, _DCH, _P, _N)
    xT = np.empty((_B, _P, 2, _DCH, _N), E4)
    xT[:, :, 0, :, :] = xTh.transpose(0, 2, 1, 3)
    xT[:, :, 1, :, :] = xTl.transpose(0, 2, 1, 3)
    # x_aug layout [b, p, t, hl, e]: x_hl[128*t + p, e], col _D = ones (hi)
    xa = np.zeros((_B, _P, _NJC, 2, _XW), E4)
    xa[:, :, :, 0, 0:_D] = xh8.reshape(_B, _NJC, _P, _D).transpose(0, 2, 1, 3)
    xa[:, :, :, 1, 0:_D] = xl8.reshape(_B, _NJC, _P, _D).transpose(0, 2, 1, 3)
    xa[:, :, :, 0, _D] = np.float32(1.0)

    in_maps = [
        {"xT": np.ascontiguousarray(xT[b]),
         "xa": np.ascontiguousarray(xa[b]),
         "P": Pp}
        for b in range(_B)
    ]
    return in_maps


def _run_on_hw(nc, in_maps, trace=False):
    from concourse import bass_utils
    from concourse.bass_interp import get_hw_module

    old_m = nc.m
    nc.m = get_hw_module(nc.m)
    try:
        res = bass_utils.run_bass_kernel_spmd(
            nc, in_maps, core_ids=list(range(len(in_maps))), trace=trace
        )
    finally:
        nc.m = old_m
    return res


def kernel(x, Wq, Wk):
    in_maps = _prep_inputs(x, Wq, Wk)
    nc = _get_nc()
    res = _run_on_hw(nc, in_maps)
    out = np.stack([res.results[b]["out"] for b in range(_B)], axis=0)
    return np.ascontiguousarray(out.astype(np.float32))


# revision 41
# speedup vs baseline: 1.0144x; 1.0029x over previous
"""Fused QK-linear attention kernel for 8 TRN2 NeuronCores (Bass/Tile).

Computes, per batch b (one batch per core):
    q = x @ Wq^T ; k = x @ Wk^T
    sim  = (q @ k^T) / sqrt(d)
    attn = softmax(sim, axis=-1)
    out  = attn @ x

Math used on device: sim = x @ P @ x^T with P = (Wq^T @ Wk) / sqrt(d)
(host-precomputed; every folded scale is a power of two, exact). Softmax
is computed without per-row max subtraction, but with a constant shift
folded into the exp bias (softmax is exactly shift-invariant) so exp()
stays inside fp8 range.

All big matmuls run as fp8e4m3 DoubleRow (K=256 per instruction, 0.5
cyc/row: 4x the fp32r MAC rate). fp8's ~2.2% quantization error is
suppressed below the accuracy gate by hi+lo fp8 splitting of every
matmul operand except exp(sim) itself:
    x  -> xh + xl        (host split, both fp8)
    P' = 128*P -> Ph + Pl  (host split)
    w' = x @ P'          (device, 3 DoubleRow chains/psum: xh*Ph + xh*Pl
                          + xl*Ph, exact fp32 PSUM) -> wh + wl split on DVE
    simT' = x @ w'^T     (3 chains: xh*wh + xl*wh + xh*wl) = 128*sim[j,i]
    ET  = fp8(exp(simT' * 2^-7 - 8*ln2))   (ScalarE, PSUM->SBUF)
    num = ET^T @ [xh|1|0] + ET^T @ [xl|0|0]  (col 256 = rowsum)
    out = num[:, :256] / num[:, 256]
Measured end-to-end rel err (L2) vs the fp32 reference: ~1.6e-2.

Scheduling: phase-1 psum groups are interleaved into phase 2's stream at
the DVE w-split drain pace; out-chunks of block ib-1 interleave into
block ib (software pipeline); block 3's chunk t0 accumulates
incrementally so its store launches right after the last activation,
while chunks t1-t3 trail and hide the earlier stores' DMA latency;
input DMAs are ordered by first use (DMA transfers are serial), and
output stores alternate between the HWDGE (sync) and SWDGE (pool) DMA
paths.
"""

import numpy as np

_B, _N, _D = 8, 2048, 256
_P = 128
_NJC = _N // _P        # 16 chunks of 128 along sequence (j)
_DCH = _D // _P        # 2 chunks of 128 along feature dim
_IBLK = 512            # i-block (matmul moving free dim)
_XW = _D + 2           # x_aug width: ones col at _D, zero pad at _D+1
_NIB = _N // _IBLK     # 4
_ICH = _IBLK // _P     # 4 i-chunks of 128 per i-block
_EXPB = float(-8.0 * np.log(2.0))
_SIMSCALE = 2.0 ** -7

_nc_cache = {}


def _build_program(loop_iters: int = 1):
    from contextlib import ExitStack, nullcontext
    from concourse import bacc, tile, mybir

    f32 = mybir.dt.float32
    fp8 = mybir.dt.float8e4
    DR = mybir.MatmulPerfMode.DoubleRow
    act_exp = mybir.ActivationFunctionType.Exp

    nc = bacc.Bacc("TRN2", debug=False, enable_asserts=True, num_devices=_B)
    # inputs pre-arranged on host to SBUF layouts (partition-major, hi/lo
    # packed into one tensor each so one DMA covers both). "head" packs P'
    # together with xT's i-block 0 so a single DMA (one semaphore, first in
    # the serial DMA queue) unblocks all of phase 1's first group.
    _HW = _D + _IBLK
    head_d = nc.dram_tensor("head", [_P, 2, _DCH, _HW], fp8,
                            kind="ExternalInput").ap()
    xT_d = nc.dram_tensor("xT", [_P, 2, _DCH, _N], fp8, kind="ExternalInput").ap()
    xa_d = nc.dram_tensor("xa", [_P, _NJC, 2, _XW], fp8, kind="ExternalInput").ap()
    out_d = nc.dram_tensor("out", [_N, _D], f32, kind="ExternalOutput").ap()

    with ExitStack() as ctx:
        tc = ctx.enter_context(tile.TileContext(nc))
        consts = ctx.enter_context(tc.tile_pool(name="consts", bufs=1))
        etp = ctx.enter_context(tc.tile_pool(name="et", bufs=2))
        outp = ctx.enter_context(tc.tile_pool(name="outsb", bufs=4))
        smallp = ctx.enter_context(tc.tile_pool(name="small", bufs=4))
        # pss: 2-bank psums for phase-2 jc-pairs; pso: 1-bank psums for
        # warmup + phase-1 groups + phase-3 out chunks
        pss = ctx.enter_context(tc.tile_pool(name="pss", bufs=2, space="PSUM"))
        pso = ctx.enter_context(tc.tile_pool(name="pso", bufs=4, space="PSUM"))

        head_sb = consts.tile([_P, 2, _DCH, _HW], fp8)
        xT_sb = consts.tile([_P, 2, _DCH, _N], fp8)
        xa_sb = consts.tile([_P, _NJC, 2, _XW], fp8)

        def P_ap(hl, esl):
            return head_sb[:, hl, :, esl]

        def xt_ap(hl, sl):
            # xT block 0 lives in the head tile (offset by _D), blocks 1-3
            # in xT_sb; every matmul slice stays within one block.
            if sl.stop <= _IBLK:
                return head_sb[:, hl, :, _D + sl.start:_D + sl.stop]
            return xT_sb[:, hl, :, sl]
        wh_sb = consts.tile([_P, _DCH, _N], fp8)
        wl_sb = consts.tile([_P, _DCH, _N], fp8)
        bias_sb = consts.tile([_P, 1], f32)
        nc.vector.memset(bias_sb, _EXPB)

        # PE warmup while the input DMAs are in flight: keeps the PE busy from
        # t=0 and gets the HAM clock gate to 2.4 GHz before real work starts.
        warm = consts.tile([_P, 2 * _P], mybir.dt.bfloat16)
        nc.gpsimd.memset(warm, 0.0)

        # DMA_ENGINES is effectively serial across transfers: order inputs by
        # first use (head = P + xT block 0, xT blocks 1-2, xa which isn't
        # needed until the first out-chunk at ~10us, then xT block 3 which
        # phase 1's last group uses at ~12us).
        nc.sync.dma_start(out=head_sb, in_=head_d)
        for nb in range(1, _NIB):
            sl = slice(nb * _IBLK, (nb + 1) * _IBLK)
            nc.sync.dma_start(out=xT_sb[:, :, :, sl], in_=xT_d[:, :, :, sl])
            if nb == 2:
                nc.sync.dma_start(out=xa_sb, in_=xa_d)

        # A couple of tiny matmuls anchor the PE clock-ramp timer early; the
        # ramp completes on wall-time, so no long busy-warmup is needed.
        for _ in range(3):
            ps = pso.tile([_P, _IBLK], f32, tag="po")
            nc.tensor.matmul(out=ps[:, 0:2 * _P], lhsT=warm[:, 0:_P], rhs=warm,
                             start=True, stop=True)

        loop_cm = tc.For_i(0, loop_iters, 1) if loop_iters > 1 else nullcontext()
        ctx.enter_context(loop_cm)

        def p1_group(nb, use_pss=False):
            """w'[e, i-block nb]: one 1-bank psum per ec chunk; wh+wl split
            on DVE (copies first so phase 2's wh-chains unblock early).
            use_pss places both ec halves in one 2-bank pss tile (for the
            late group, once the pso pool is serving out-chunks)."""
            isl = slice(nb * _IBLK, (nb + 1) * _IBLK)
            pshalf = []
            big = None
            if use_pss:
                big = pss.tile([_P, 2 * _IBLK], f32, tag="sim", name="p1big")
            for ec in range(_DCH):
                esl = slice(ec * _P, (ec + 1) * _P)
                if use_pss:
                    ps = big[:, ec * _IBLK:(ec + 1) * _IBLK]
                else:
                    ps = pso.tile([_P, _IBLK], f32, tag="po")
                pshalf.append(ps)
                nc.tensor.matmul(out=ps, lhsT=P_ap(0, esl),
                                 rhs=xt_ap(0, isl), start=True,
                                 stop=False, perf_mode=DR)
                nc.tensor.matmul(out=ps, lhsT=P_ap(1, esl),
                                 rhs=xt_ap(0, isl), start=False,
                                 stop=False, perf_mode=DR)
                nc.tensor.matmul(out=ps, lhsT=P_ap(0, esl),
                                 rhs=xt_ap(1, isl), start=False,
                                 stop=True, perf_mode=DR)
                nc.vector.tensor_copy(out=wh_sb[:, ec, isl], in_=ps)
            for ec in range(_DCH):
                nc.vector.tensor_sub(out=wl_sb[:, ec, isl], in0=pshalf[ec],
                                     in1=wh_sb[:, ec, isl])

        out_r = out_d.rearrange("(g p) d -> p g d", p=_P)

        def p2_pair(ib, jp, et):
            """simT' + exp for j-chunks (2jp, 2jp+1) of i-block ib."""
            isl = slice(ib * _IBLK, (ib + 1) * _IBLK)
            ps = pss.tile([_P, 2 * _IBLK], f32, tag="sim")
            for h in range(2):
                jsl = slice((2 * jp + h) * _P, (2 * jp + h + 1) * _P)
                half = ps[:, h * _IBLK:(h + 1) * _IBLK]
                nc.tensor.matmul(out=half, lhsT=xt_ap(0, jsl),
                                 rhs=wh_sb[:, :, isl], start=True,
                                 stop=False, perf_mode=DR)
                nc.tensor.matmul(out=half, lhsT=xt_ap(1, jsl),
                                 rhs=wh_sb[:, :, isl], start=False,
                                 stop=False, perf_mode=DR)
                nc.tensor.matmul(out=half, lhsT=xt_ap(0, jsl),
                                 rhs=wl_sb[:, :, isl], start=False,
                                 stop=True, perf_mode=DR)
            nc.scalar.activation(out=et[:, 2 * jp:2 * jp + 2, :], in_=ps,
                                 func=act_exp, scale=_SIMSCALE, bias=bias_sb)

        def chunk_dr_pair(et, po, t, u):
            """the j-pair-u contribution (hi + lo chains) to chunk t's psum."""
            poa = po[:, 0:_XW]
            tsl = slice(t * _P, (t + 1) * _P)
            nc.tensor.matmul(out=poa, lhsT=et[:, 2 * u:2 * u + 2, tsl],
                             rhs=xa_sb[:, 2 * u:2 * u + 2, 0, :],
                             start=(u == 0), stop=False, perf_mode=DR)
            nc.tensor.matmul(out=poa, lhsT=et[:, 2 * u:2 * u + 2, tsl],
                             rhs=xa_sb[:, 2 * u:2 * u + 2, 1, :],
                             start=False, stop=(u == _NJC // 2 - 1),
                             perf_mode=DR)

        act_copy = mybir.ActivationFunctionType.Copy

        def finish_chunk(ib, po, t, scalar_mul=False):
            """normalize chunk t of block ib and store it. scalar_mul moves
            the multiply to ScalarE (idle at the tail) so the last chunks'
            normalizations pipeline instead of serializing on DVE."""
            poa = po[:, 0:_XW]
            recip = smallp.tile([_P, 1], f32)
            nc.vector.reciprocal(out=recip, in_=poa[:, _D:_D + 1])
            o_t = outp.tile([_P, _D], f32)
            if scalar_mul:
                nc.scalar.activation(out=o_t, in_=poa[:, 0:_D], func=act_copy,
                                     scale=recip)
            else:
                nc.vector.tensor_scalar_mul(out=o_t, in0=poa[:, 0:_D],
                                            scalar1=recip)
            chunk = ib * _ICH + t
            eng = nc.sync if chunk % 2 == 0 else nc.gpsimd
            eng.dma_start(out=out_r[:, chunk, :], in_=o_t)

        def out_chunk(ib, et, t):
            """one i-chunk of the numerator/rowsum + normalize + store."""
            po = pso.tile([_P, _IBLK], f32, tag="po")
            for u in range(_NJC // 2):
                chunk_dr_pair(et, po, t, u)
            finish_chunk(ib, po, t)

        # Software pipeline: p1(nb0/nb1) prologue, p1(nb2) early in block 0,
        # p1(nb3) after block 1 (so its DVE ops queue AFTER block-0 norms);
        # out-chunks of block ib-1 interleave into block ib; tail runs block
        # 3's out-chunks.
        p1_group(0)
        p1_group(1)
        prev = None
        for ib in range(_NIB - 1):
            et = etp.tile([_P, _NJC, _IBLK], fp8, tag="et")
            for jp in range(_NJC // 2):
                p2_pair(ib, jp, et)
                if ib == 0:
                    if jp == 1:
                        p1_group(2)
                elif 4 <= jp <= 7:
                    pib, pet = prev
                    out_chunk(pib, pet, jp - 4)
            if ib == 1:
                p1_group(3)
            prev = (ib, et)

        # Last block: block 2's chunks run in the first half. Block 3's chunk
        # t0 accumulates incrementally through the block's second half (so its
        # store launches right after the last activation); t1-t3 run as full
        # trailing chains whose PE time hides t0-t2's normalize+DMA latency.
        pib, pet = prev
        et = etp.tile([_P, _NJC, _IBLK], fp8, tag="et")
        po0 = None
        for jp in range(_NJC // 2):
            p2_pair(_NIB - 1, jp, et)
            if jp <= 3:
                out_chunk(pib, pet, jp)
            elif jp == 4:
                po0 = pso.tile([_P, _IBLK], f32, tag="po")
                for u in range(5):
                    chunk_dr_pair(et, po0, 0, u)
            else:
                chunk_dr_pair(et, po0, 0, jp)
        finish_chunk(_NIB - 1, po0, 0, scalar_mul=True)
        for t in range(1, _ICH):
            out_chunk(_NIB - 1, et, t)

    nc.compile()
    return nc


def _get_nc(mm_dtype=None):
    key = "fp8dr"
    if key not in _nc_cache:
        _nc_cache[key] = _build_program()
    return _nc_cache[key]


def _prep_inputs(x, Wq, Wk):
    import ml_dtypes

    E4 = ml_dtypes.float8_e4m3
    x = np.asarray(x, dtype=np.float32)
    Wq = np.asarray(Wq, dtype=np.float32)
    Wk = np.asarray(Wk, dtype=np.float32)
    # P' = 128 * Wq^T @ Wk / sqrt(d); hi+lo fp8 split (host)
    P = ((Wq.astype(np.float64).T @ Wk.astype(np.float64)) * (2.0 ** -4) * 128.0
         ).astype(np.float32)
    Ph8 = P.astype(E4)
    Pl8 = (P - Ph8.astype(np.float32)).astype(E4)
    # [p, hl, dc, e]: P'_hl[128*dc + p, e]
    Pp = np.stack([Ph8.reshape(_DCH, _P, _D).transpose(1, 0, 2),
                   Pl8.reshape(_DCH, _P, _D).transpose(1, 0, 2)], axis=1)
    Pp = np.ascontiguousarray(Pp)

    xh8 = x.astype(E4)                                   # [b, n, d]
    xl8 = (x - xh8.astype(np.float32)).astype(E4)
    # xT layout [b, p, hl, dc, n]: x_hl[n, 128*dc + p]
    xTh = xh8.transpose(0, 2, 1).reshape(_B, _DCH, _P, _N)
    xTl = xl8.transpose(0, 2, 1).reshape(_B, _DCH, _P, _N)
    xT = np.empty((_B, _P, 2, _DCH, _N), E4)
    xT[:, :, 0, :, :] = xTh.transpose(0, 2, 1, 3)
    xT[:, :, 1, :, :] = xTl.transpose(0, 2, 1, 3)
    # x_aug layout [b, p, t, hl, e]: x_hl[128*t + p, e], col _D = ones (hi)
    xa = np.zeros((_B, _P, _NJC, 2, _XW), E4)
    xa[:, :, :, 0, 0:_D] = xh8.reshape(_B, _NJC, _P, _D).transpose(0, 2, 1, 3)
    xa[:, :, :, 1, 0:_D] = xl8.reshape(_B, _NJC, _P, _D).transpose(0, 2, 1, 3)
    xa[:, :, :, 0, _D] = np.float32(1.0)

    # head packs P' with xT's i-block 0: [p, hl, dc, 0:256]=P', [256:768]=xT
    head = np.concatenate([np.broadcast_to(Pp[None], (_B,) + Pp.shape),
                           xT[:, :, :, :, 0:_IBLK]], axis=4)
    in_maps = [
        {"xT": np.ascontiguousarray(xT[b]),
         "xa": np.ascontiguousarray(xa[b]),
         "head": np.ascontiguousarray(head[b])}
        for b in range(_B)
    ]
    return in_maps


def _run_on_hw(nc, in_maps, trace=False):
    from concourse import bass_utils
    from concourse.bass_interp import get_hw_module

    old_m = nc.m
    nc.m = get_hw_module(nc.m)
    try:
        res = bass_utils.run_bass_kernel_spmd(
            nc, in_maps, core_ids=list(range(len(in_maps))), trace=trace
        )
    finally:
        nc.m = old_m
    return res


def kernel(x, Wq, Wk):
    in_maps = _prep_inputs(x, Wq, Wk)
    nc = _get_nc()
    res = _run_on_hw(nc, in_maps)
    out = np.stack([res.results[b]["out"] for b in range(_B)], axis=0)
    return np.ascontiguousarray(out.astype(np.float32))
